# revision 1
# baseline (speedup 1.0000x reference)
"""GCN-Tox21 GNN message-passing kernel for 8 Trainium2 NeuronCores.

Strategy (graph/edge parallelism):
  - Sort edges by destination node on the host; core k owns the destination
    node range [k*NPC, (k+1)*NPC) and all edges pointing into it.
  - Node features h live replicated in each core's DRAM (bf16). Per-edge
    gathers of h[dst], h[src] use dma_gather(transpose=True), which lands
    features feature-major in SBUF, ready as matmul moving operands.
  - The per-edge 2-layer MLP runs on the tensor engine in bf16 with fp32
    PSUM accumulation. b1 is folded in via a constant-ones row appended to
    the e^T operand; b2 is added with a broadcast tile on the vector engine.
  - Segment-sum to destination nodes is a matmul with host-built 0/1 one-hot
    tiles (edges sorted by dst => each 128-node window's edges are
    contiguous; PSUM accumulates across the window's edge subtiles).
  - Mean + eval-mode BN fold into h = relu((seg_sum + cnt'*cb) * invcnt),
    cb = bn_b - bn_m*A, A = g/sqrt(rv+eps); w2/b2 pre-scaled by A. The
    rank-1 cnt'*cb term is one K=1 matmul per window.
  - After each conv layer an AllGather rebuilds the replicated h.
  - Mean-pool + FC + sigmoid: core k handles graphs [k*GPC, (k+1)*GPC)
    (batch is sorted, so their nodes are contiguous; dma_gather fetches
    them node-major for the pooling matmul).
"""

import numpy as np
import ml_dtypes

import concourse.bacc as bacc
import concourse.tile as tile
from concourse import mybir, bass_utils
from concourse.masks import make_identity

BF16 = mybir.dt.bfloat16
F32 = mybir.dt.float32
I16 = mybir.dt.int16
RELU = mybir.ActivationFunctionType.Relu

N_CORES = 8
BN_EPS = 1e-5
G_REAL = 512
F_NODE, F_EDGE, H, EH = 32, 8, 256, 16
OUT_DIMS = (256, 256, 128)
EG = 512  # edges per gather batch


def _bf(a):
    return np.ascontiguousarray(a.astype(ml_dtypes.bfloat16))


def _f32(a):
    return np.ascontiguousarray(a.astype(np.float32))


def _wrap_idx(idx):
    """int16 index layout for dma_gather: index i at [i % 16, i // 16],
    replicated across the 8 partition groups."""
    assert len(idx) % 16 == 0
    w = idx.astype(np.int16).reshape(-1, 16).T
    return np.ascontiguousarray(np.tile(w, (8, 1)))


class Plan:
    """Host-side preprocessing: sharding layout + per-core input tensors."""

    def __init__(self, inputs, G):
        x = np.asarray(inputs["x"]).astype(np.float32)
        N = x.shape[0]
        self.N, self.G = N, G
        self.N_pad = ((N + N_CORES * 128 - 1) // (N_CORES * 128)) * (N_CORES * 128)
        self.NPC = self.N_pad // N_CORES
        self.W = self.NPC // 128
        assert G % N_CORES == 0
        self.GPC = G // N_CORES

        edge_index = np.asarray(inputs["edge_index"]).astype(np.int64)
        src, dst = edge_index[0].astype(np.int32), edge_index[1].astype(np.int32)
        batch = np.asarray(inputs["batch"]).astype(np.int32)
        edge_attr = np.asarray(inputs["edge_attr"]).astype(np.float32)

        order = np.argsort(dst, kind="stable")
        s_dst, s_src = dst[order], src[order]
        s_ea = edge_attr[order]

        bounds = np.searchsorted(s_dst, np.arange(0, self.N_pad + 1, 128), "left")
        cnt_w = (bounds[1:] - bounds[:-1]).reshape(N_CORES, self.W)
        T_w = np.maximum(1, -(-cnt_w.max(axis=0) // 128))
        while T_w.sum() % (EG // 128) != 0:
            T_w[-1] += 1
        self.T_w = [int(t) for t in T_w]
        self.T_tot = int(T_w.sum())
        self.ET = self.T_tot * 128

        cnt = np.bincount(dst, minlength=self.N_pad).astype(np.float32)
        invc_full = 1.0 / np.maximum(cnt, 1.0)
        cntp_full = np.maximum(cnt, 1.0)
        gcnt = np.bincount(batch, minlength=G).astype(np.float32)
        ginv_full = 1.0 / np.maximum(gcnt, 1.0)

        lo_k = [int(np.searchsorted(batch, k * self.GPC, "left")) for k in range(N_CORES)]
        hi_k = [int(np.searchsorted(batch, (k + 1) * self.GPC, "left")) for k in range(N_CORES)]
        self.TP = max(1, max(-(-(h - l) // 128) for l, h in zip(lo_k, hi_k)))
        self.NPOOL = self.TP * 128

        self.per_core = []
        for k in range(N_CORES):
            d = {}
            gi_dst = np.zeros(self.ET, np.int32)
            gi_src = np.zeros(self.ET, np.int32)
            ea_pad = np.zeros((self.ET, F_EDGE), np.float32)
            S = np.zeros((128, self.ET), np.float32)
            pos = 0
            for w in range(self.W):
                base = k * self.NPC + w * 128
                lo = np.searchsorted(s_dst, base, "left")
                hi = np.searchsorted(s_dst, base + 128, "left")
                n = hi - lo
                sl = slice(pos, pos + n)
                gi_dst[sl] = s_dst[lo:hi]
                gi_src[sl] = s_src[lo:hi]
                ea_pad[sl] = s_ea[lo:hi]
                loc = (s_dst[lo:hi] - base).astype(np.int64)
                e_ids = np.arange(pos, pos + n)
                S[e_ids % 128, (e_ids // 128) * 128 + loc] = 1.0
                pos += self.T_w[w] * 128
            assert pos == self.ET

            d["gidx_src"] = _wrap_idx(gi_src)
            d["S"] = _bf(S)
            # transposed one-hot: S_T[n, t*128+p] = S[p, t*128+n]
            ST = np.ascontiguousarray(
                S.reshape(128, self.T_tot, 128).transpose(2, 1, 0)
                .reshape(128, self.ET))
            d["S_T"] = _bf(ST)
            eaT = np.concatenate([ea_pad.T, np.ones((1, self.ET), np.float32)], 0)
            d["eaT"] = _bf(eaT)
            stripe = slice(k * self.NPC, (k + 1) * self.NPC)
            d["invc"] = _f32(invc_full[stripe].reshape(self.W, 128).T)
            d["cntrow"] = _bf(cntp_full[stripe].reshape(1, self.NPC))
            lo, hi = lo_k[k], hi_k[k]
            pidx = np.zeros(self.NPOOL, np.int32)
            pidx[: hi - lo] = np.arange(lo, hi)
            d["pool_idx"] = _wrap_idx(pidx)
            S2 = np.zeros((128, self.TP * self.GPC), np.float32)
            pb = batch[lo:hi] - k * self.GPC
            e_ids = np.arange(hi - lo)
            S2[e_ids % 128, (e_ids // 128) * self.GPC + pb] = 1.0
            d["S2"] = _bf(S2)
            d["ginv"] = _f32(ginv_full[k * self.GPC:(k + 1) * self.GPC].reshape(self.GPC, 1))
            self.per_core.append(d)

        sh = {}
        x_pad = np.zeros((self.N_pad, F_NODE), np.float32)
        x_pad[:N] = x
        xT_full = np.concatenate([x_pad.T, np.ones((1, self.N_pad), np.float32)], 0)
        sh["xT"] = _bf(xT_full)
        for k in range(N_CORES):
            self.per_core[k]["xT_own"] = _bf(
                xT_full[:, k * self.NPC:(k + 1) * self.NPC])
        ne_w, ne_b = _f32(inputs["ne_w"]), _f32(inputs["ne_b"])
        sh["ne_wT"] = _bf(np.concatenate([ne_w.T, ne_b[None, :]], 0))
        ee_w, ee_b = _f32(inputs["ee_w"]), _f32(inputs["ee_b"])
        sh["ee_wT"] = _bf(np.concatenate([ee_w.T, ee_b[None, :]], 0))

        in_dim = H
        self.layer_dims = []
        for i, out_dim in enumerate(OUT_DIMS):
            w1 = _f32(inputs[f"c{i}_w1"]); b1 = _f32(inputs[f"c{i}_b1"])
            w2 = _f32(inputs[f"c{i}_w2"]); b2 = _f32(inputs[f"c{i}_b2"])
            g = _f32(inputs[f"bn{i}_g"]); bb = _f32(inputs[f"bn{i}_b"])
            rm = _f32(inputs[f"bn{i}_m"]); rv = _f32(inputs[f"bn{i}_v"])
            A = g / np.sqrt(rv + BN_EPS)
            F_mid = 2 * out_dim
            # K-order: [h_dst(in), h_src(in), e(EH), ones]
            sh[f"w1T_{i}"] = _bf(np.concatenate([w1.T, b1[None, :]], 0))
            sh[f"w2T_{i}"] = _bf((w2 * A[:, None]).T)
            sh[f"b2bc_{i}"] = _f32(np.tile((b2 * A)[None, :], (128, 1)))
            sh[f"cbrow_{i}"] = _bf((bb - rm * A)[None, :])
            self.layer_dims.append((in_dim, F_mid, out_dim))
            in_dim = out_dim

        fc_w, fc_b = _f32(inputs["fc_w"]), _f32(inputs["fc_b"])
        self.F_FC = fc_w.shape[0]
        sh["fc_wT"] = _bf(fc_w.T)
        sh["fcb_bc"] = _f32(np.tile(fc_b[None, :], (self.GPC, 1)))
        self.shared = sh

    def in_maps(self):
        return [{**self.shared, **self.per_core[k]} for k in range(N_CORES)]


def build_program(plan: Plan, n_cores=N_CORES, debug_no_collective=False,
                  debug_stage=9, repeats=1, skip_gather=False, skip_compute=False):
    nc = bacc.Bacc("TRN2", target_bir_lowering=False, debug=False,
                   num_devices=n_cores)

    ET, T_w, W, NPC, TP, GPC = plan.ET, plan.T_w, plan.W, plan.NPC, plan.TP, plan.GPC
    N_pad, NPOOL, F_FC = plan.N_pad, plan.NPOOL, plan.F_FC

    sample = plan.in_maps()[0]
    t_in = {name: nc.dram_tensor(name, list(arr.shape),
                                 mybir.dt.from_np(arr.dtype), kind="ExternalInput")
            for name, arr in sample.items()}
    out_part = nc.dram_tensor("out_part", [GPC, F_FC], F32, kind="ExternalOutput")

    n_batches = ET // EG if ET % EG == 0 else ET // EG + 1

    # subtile -> window mapping (static)
    sub_window, sub_first, sub_last = [], [], []
    for w in range(W):
        for t in range(T_w[w]):
            sub_window.append(w)
            sub_first.append(t == 0)
            sub_last.append(t == T_w[w] - 1)

    with tile.TileContext(nc) as tc:
        with (
            tc.tile_pool(name="const", bufs=1) as cpool,
            tc.tile_pool(name="sbuf", bufs=2) as spool,
            tc.tile_pool(name="gath", bufs=8) as gpool,
            tc.tile_pool(name="m1sb", bufs=8) as m1pool,
            tc.tile_pool(name="psum", bufs=2, space="PSUM") as ppool,
            tc.tile_pool(name="dram", bufs=1, space="DRAM") as dpool,
        ):
            def _body():
                # ---------- resident constants ----------
                def load_const(name, dtype=None, tag=None):
                    arr = sample[name]
                    t = cpool.tile(list(arr.shape), dtype or mybir.dt.from_np(arr.dtype),
                                   tag=tag or name)
                    nc.sync.dma_start(out=t[:], in_=t_in[name][:])
                    return t

                ST_t = load_const("S_T")
                gidx_src = load_const("gidx_src")
                pool_idx = load_const("pool_idx")
                invc_t = load_const("invc")
                cntrow_t = load_const("cntrow")
                S2_t = load_const("S2")
                ginv_t = load_const("ginv")
                ne_wT_t = load_const("ne_wT")
                ee_wT_t = load_const("ee_wT")
                fc_wT_t = load_const("fc_wT")
                fcb_t = load_const("fcb_bc")
                ident = cpool.tile([128, 128], BF16, tag="ident")
                make_identity(nc, ident[:])

                w1T_t, w2T_t, b2bc_t, cbrow_t = [], [], [], []
                for i, (F_in, F_mid, F_out) in enumerate(plan.layer_dims):
                    KC2 = 2 * F_in // 128
                    chunks = []
                    for kc in range(KC2):
                        t = cpool.tile([128, F_mid], BF16, tag=f"w1T_{i}_{kc}")
                        nc.sync.dma_start(out=t[:],
                                          in_=t_in[f"w1T_{i}"][kc * 128:(kc + 1) * 128, :])
                        chunks.append(t)
                    te = cpool.tile([EH + 1, F_mid], BF16, tag=f"w1Te_{i}")
                    nc.sync.dma_start(out=te[:],
                                      in_=t_in[f"w1T_{i}"][2 * F_in:2 * F_in + EH + 1, :])
                    w1T_t.append((chunks, te))
                    wc = []
                    for km in range(F_mid // 128):
                        t = cpool.tile([128, F_out], BF16, tag=f"w2T_{i}_{km}")
                        nc.sync.dma_start(out=t[:],
                                          in_=t_in[f"w2T_{i}"][km * 128:(km + 1) * 128, :])
                        wc.append(t)
                    w2T_t.append(wc)
                    b2bc_t.append(load_const(f"b2bc_{i}"))
                    cbrow_t.append(load_const(f"cbrow_{i}"))
                hT_t = [cpool.tile([128, W, F_in_ // 128, 128], BF16, tag=f"hT{i}",
                                   name=f"hT{i}")
                        for i, (F_in_, _, _) in enumerate(plan.layer_dims)]

                # ---------- DRAM buffers ----------
                h_full = [dpool.tile([N_pad, plan.layer_dims[0][0]], BF16, tag="h0",
                                     name="h_full0")]
                for i, (_, _, F_out) in enumerate(plan.layer_dims):
                    h_full.append(dpool.tile([N_pad, F_out], BF16, tag=f"h{i + 1}",
                                             name=f"h_full{i + 1}"))
                h_own = [dpool.tile([NPC, d[2]], BF16, tag=f"hown{i}",
                                    name=f"h_own{i}")
                         for i, d in enumerate(plan.layer_dims)]
                eT_dram = dpool.tile([EH + 1, ET], BF16, tag="eT")

                # ---------- stage A: h0 = relu(x @ ne_w.T + ne_b) (replicated) ----------
                for chunk in range(N_pad // 128):
                    n0 = chunk * 128
                    xt = spool.tile([F_NODE + 1, 128], BF16, tag="xT")
                    nc.sync.dma_start(out=xt[:], in_=t_in["xT"][:, n0:n0 + 128])
                    ps = ppool.tile([128, H], F32, tag="m2")
                    nc.tensor.matmul(out=ps[:], lhsT=xt[:], rhs=ne_wT_t[:],
                                     start=True, stop=True)
                    h0sb = spool.tile([128, H], BF16, tag="h0sb")
                    nc.scalar.activation(out=h0sb[:], in_=ps[:], func=RELU)
                    nc.sync.dma_start(out=h_full[0][n0:n0 + 128, :], in_=h0sb[:])
                # own-stripe h0 again, transposed into hT_t[0] for the dst path
                for w in range(W):
                    xo = spool.tile([F_NODE + 1, 128], BF16, tag="xT")
                    nc.sync.dma_start(out=xo[:],
                                      in_=t_in["xT_own"][:, w * 128:(w + 1) * 128])
                    ps = ppool.tile([128, H], F32, tag="m2")
                    nc.tensor.matmul(out=ps[:], lhsT=xo[:], rhs=ne_wT_t[:],
                                     start=True, stop=True)
                    h0o = spool.tile([128, H], BF16, tag="h0sb")
                    nc.scalar.activation(out=h0o[:], in_=ps[:], func=RELU)
                    for kc in range(H // 128):
                        tp = ppool.tile([128, 128], BF16, tag="m2")
                        nc.tensor.transpose(out=tp[:],
                                            in_=h0o[:, kc * 128:(kc + 1) * 128],
                                            identity=ident[:])
                        nc.vector.tensor_copy(out=hT_t[0][:, w, kc, :], in_=tp[:])

                # ---------- stage A2: e^T (+ones row) -> DRAM [EH+1, ET] ----------
                if debug_stage < 2:
                    return
                ones_row = cpool.tile([1, 512], BF16, tag="ones512")
                nc.vector.memset(ones_row[:], 1.0)
                for g0 in range(ET // 512):
                    ea_t = spool.tile([F_EDGE + 1, 512], BF16, tag="eaT")
                    nc.sync.dma_start(out=ea_t[:],
                                      in_=t_in["eaT"][:, g0 * 512:(g0 + 1) * 512])
                    ps = ppool.tile([EH, 512], F32, tag="m2")
                    nc.tensor.matmul(out=ps[:], lhsT=ee_wT_t[:], rhs=ea_t[:],
                                     start=True, stop=True)
                    et_sb = spool.tile([EH, 512], BF16, tag="etsb")
                    nc.scalar.activation(out=et_sb[:], in_=ps[:], func=RELU)
                    nc.sync.dma_start(out=eT_dram[0:EH, g0 * 512:(g0 + 1) * 512],
                                      in_=et_sb[:])
                    nc.sync.dma_start(out=eT_dram[EH:EH + 1, g0 * 512:(g0 + 1) * 512],
                                      in_=ones_row[:])

                # ---------- conv layers ----------
                if debug_stage < 3:
                    return
                for li, (F_in, F_mid, F_out) in enumerate(plan.layer_dims):
                    h_in = h_full[li]
                    KC = F_in // 128
                    MC = F_mid // 128
                    node_ps = None
                    qt_by_window = {}
                    for b in range(n_batches):
                        e0 = b * EG
                        eg = min(EG, ET - e0)
                        gs = gpool.tile([128, KC, eg], BF16, tag="gs")
                        if not skip_gather:
                            nc.gpsimd.dma_gather(gs[:], h_in[:, :],
                                                 gidx_src[:, e0 // 16:(e0 + eg) // 16],
                                                 eg, eg, F_in, transpose=True)
                        for gsub in (range(0) if skip_compute else range(eg // 512)):
                            g = (e0 + gsub * 512) // 512
                            c0 = gsub * 512
                            et_t = spool.tile([EH + 1, 512], BF16, tag="et_in", bufs=4)
                            nc.sync.dma_start(out=et_t[:],
                                              in_=eT_dram[:, g * 512:(g + 1) * 512])
                            # per-window Q^T = h_win @ W1d.T, expanded per edge below
                            for s in range(4):
                                t_glob = g * 4 + s
                                if sub_first[t_glob]:
                                    w = sub_window[t_glob]
                                    qtp = ppool.tile([128, F_mid], F32, tag="qt")
                                    for kc in range(KC):
                                        nc.tensor.matmul(
                                            out=qtp[:], lhsT=hT_t[li][:, w, kc, :],
                                            rhs=w1T_t[li][0][kc][:],
                                            start=(kc == 0), stop=(kc == KC - 1))
                                    qsb = spool.tile([128, F_mid], BF16, tag="qt_sb",
                                                     bufs=3)
                                    nc.vector.tensor_copy(out=qsb[:], in_=qtp[:])
                                    qt_by_window[w] = qsb
                            m1sb = []
                            for fo in range(MC):
                                ps = ppool.tile([128, 512], F32, tag="m1")
                                fsl = slice(fo * 128, (fo + 1) * 128)
                                for kc in range(KC):
                                    nc.tensor.matmul(
                                        out=ps[:], lhsT=w1T_t[li][0][KC + kc][:, fsl],
                                        rhs=gs[:, kc, c0:c0 + 512],
                                        start=(kc == 0), stop=False,
                                        skip_group_check=True)
                                s = 0
                                while s < 4:
                                    t_glob = g * 4 + s
                                    w0 = sub_window[t_glob]
                                    s2 = s
                                    while s2 + 1 < 4 and sub_window[g * 4 + s2 + 1] == w0:
                                        s2 += 1
                                    qsb = qt_by_window[w0]
                                    nc.tensor.matmul(
                                        out=ps[:, s * 128:(s2 + 1) * 128],
                                        lhsT=qsb[:, fsl],
                                        rhs=ST_t[:, t_glob * 128:
                                                 (g * 4 + s2 + 1) * 128],
                                        start=False, stop=False,
                                        skip_group_check=True)
                                    s = s2 + 1
                                nc.tensor.matmul(
                                    out=ps[:], lhsT=w1T_t[li][1][:, fsl], rhs=et_t[:],
                                    start=False, stop=True, skip_group_check=True)
                                sb = m1pool.tile([128, 512], BF16, tag="m1sb")
                                nc.scalar.activation(out=sb[:], in_=ps[:], func=RELU)
                                m1sb.append(sb)
                            for s in range(4):
                                t_glob = g * 4 + s
                                w = sub_window[t_glob]
                                ps2 = ppool.tile([128, F_out], F32, tag="m2")
                                esl = slice(s * 128, (s + 1) * 128)
                                for km in range(MC):
                                    nc.tensor.matmul(
                                        out=ps2[:], lhsT=m1sb[km][:, esl],
                                        rhs=w2T_t[li][km][:],
                                        start=(km == 0), stop=(km == MC - 1))
                                m2sb = spool.tile([128, F_out], BF16, tag="m2sb")
                                nc.vector.tensor_tensor(out=m2sb[:], in0=ps2[:],
                                                        in1=b2bc_t[li][:],
                                                        op=mybir.AluOpType.add)
                                st_t = spool.tile([128, 128], BF16, tag="s_in",
                                                  bufs=6)
                                nc.sync.dma_start(
                                    out=st_t[:],
                                    in_=t_in["S"][:, t_glob * 128:(t_glob + 1) * 128])
                                if sub_first[t_glob]:
                                    node_ps = ppool.tile([128, F_out], F32, tag="node")
                                    nc.tensor.matmul(
                                        out=node_ps[:],
                                        lhsT=cntrow_t[0:1, w * 128:(w + 1) * 128],
                                        rhs=cbrow_t[li][:], start=True, stop=False,
                                        skip_group_check=True)
                                nc.tensor.matmul(
                                    out=node_ps[:],
                                    lhsT=st_t[:],
                                    rhs=m2sb[:], start=False, stop=sub_last[t_glob],
                                    skip_group_check=True)
                                if sub_last[t_glob]:
                                    hsb = spool.tile([128, F_out], BF16, tag="hsb")
                                    nc.scalar.activation(out=hsb[:], in_=node_ps[:],
                                                         func=RELU,
                                                         scale=invc_t[:, w:w + 1])
                                    nc.sync.dma_start(
                                        out=h_own[li][w * 128:(w + 1) * 128, :],
                                        in_=hsb[:])
                                    if li < 2:
                                        for kc in range(F_out // 128):
                                            tp = ppool.tile([128, 128], BF16,
                                                            tag="m2")
                                            nc.tensor.transpose(
                                                out=tp[:],
                                                in_=hsb[:, kc * 128:(kc + 1) * 128],
                                                identity=ident[:])
                                            nc.vector.tensor_copy(
                                                out=hT_t[li + 1][:, w, kc, :],
                                                in_=tp[:])
                    if debug_no_collective:
                        cp = spool.tile([128, F_out], BF16, tag="dbgcp")
                        nc.sync.dma_start(out=cp[:], in_=h_own[li][0:128, :])
                        nc.sync.dma_start(out=h_full[li + 1][0:128, :], in_=cp[:])
                    else:
                        nc.gpsimd.collective_compute(
                            "AllGather", mybir.AluOpType.bypass,
                            ins=[h_own[li].opt()], outs=[h_full[li + 1].opt()],
                            replica_groups=[list(range(n_cores))])

                # ---------- pooling + FC + sigmoid ----------
                if debug_stage < 5:
                    return
                F_last = plan.layer_dims[-1][2]
                hp = spool.tile([128, TP, F_last], BF16, tag="hp")
                # gather in <=512-index chunks (larger single gathers crash)
                for p0 in range(0, TP, 4):
                    pn = min(4, TP - p0)
                    nc.gpsimd.dma_gather(
                        hp[:, p0:p0 + pn, :], h_full[-1][:, :],
                        pool_idx[:, p0 * 8:(p0 + pn) * 8],
                        pn * 128, pn * 128, F_last, transpose=False)
                if debug_stage < 6:
                    return
                pool_ps = ppool.tile([GPC, F_last], F32, tag="m1")
                for t in range(TP):
                    nc.tensor.matmul(out=pool_ps[:],
                                     lhsT=S2_t[:, t * GPC:(t + 1) * GPC],
                                     rhs=hp[:, t, :], start=(t == 0), stop=(t == TP - 1))
                pooled_sb = spool.tile([GPC, F_last], BF16, tag="pooled")
                nc.scalar.activation(out=pooled_sb[:], in_=pool_ps[:],
                                     func=mybir.ActivationFunctionType.Copy,
                                     scale=ginv_t[:])
                if debug_stage < 7:
                    return
                ptr_ps = ppool.tile([F_last, GPC], BF16, tag="qt")
                nc.tensor.transpose(out=ptr_ps[:], in_=pooled_sb[:],
                                    identity=ident[0:GPC, 0:GPC])
                ptr_sb = spool.tile([F_last, GPC], BF16, tag="ptrsb")
                nc.vector.tensor_copy(out=ptr_sb[:], in_=ptr_ps[:])
                if debug_stage < 8:
                    return
                fc_ps = ppool.tile([GPC, F_FC], F32, tag="node")
                nc.tensor.matmul(out=fc_ps[:], lhsT=ptr_sb[:], rhs=fc_wT_t[:],
                                 start=True, stop=True)
                logit = spool.tile([GPC, F_FC], F32, tag="logit")
                nc.vector.tensor_tensor(out=logit[:], in0=fc_ps[:], in1=fcb_t[:],
                                        op=mybir.AluOpType.add)
                # Sigmoid's activation-table load (sigmoid_and_friends) crashes
                # this runtime; the host applies the exact fp32 sigmoid instead.
                nc.sync.dma_start(out=out_part[:], in_=logit[:])

            for _r in range(repeats):
                _body()

    nc.compile()
    return nc


_CACHE = {}


def run(inputs, G=G_REAL):
    plan = Plan(inputs, G)
    key = (plan.N, plan.G, plan.TP, tuple(plan.T_w))
    if key not in _CACHE:
        _CACHE[key] = build_program(plan)
    nc = _CACHE[key]
    res = bass_utils.run_bass_kernel_spmd(nc, plan.in_maps(),
                                          core_ids=list(range(N_CORES)))
    logits = np.concatenate([res.results[k]["out_part"] for k in range(N_CORES)], 0)
    out = 1.0 / (1.0 + np.exp(-logits.astype(np.float64)))
    return np.ascontiguousarray(out.astype(np.float32))


def kernel(**inputs) -> np.ndarray:
    return run(inputs, G=G_REAL)

